# revision 85
# baseline (speedup 1.0000x reference)
"""GQA attention block (B=2, T=2048, D=2048, 16 Q heads, 4 KV heads, RoPE,
causal, out-projection) on 8 Trainium2 NeuronCores.

Sharding: core i = (batch b = i//4, kv-group g = i%4). Each core computes the
4 query heads of its kv-group for its batch, then a partial output projection
with the matching 512 rows of wo; the host sums the 4 partials per batch.

Design (bf16 matmuls, software-pipelined; ~217us/core in TimelineSim, from a
410us f32r baseline):
  - All matmul operands bf16 (1 cycle/row on PE, FWL weight loads), PSUM f32.
  - Software pipeline at 512-token-block (tb) granularity: projections for
    block tb are emitted before attention for block tb-1, so the PE queue
    always holds independent projection matmuls while the DVE/ACT softmax
    chain of the previous block drains. Output-projection matmul groups of
    block tb-1 are interleaved into the attention stream of block tb at
    4-matmul granularity to fill exp-latency bubbles.
  - Projections: lhsT = x^T chunk (stationary), rhs = [wq|wk|wv] columns.
    RoPE: pq/pkv staged PSUM->SBUF bf16 on ACT (frees PSUM early), rotation
    on DVE at 2x bf16 rate. PE transposes (bf16, one identity matmul per
    128-slice) deferred one chunk behind the projection matmuls so rope
    latency is hidden; one strided ACT copy scatters all 4 heads into the
    per-block Q^T tile.
  - Causal handling is exact at 128-block granularity: diagonal strips only
    compute t >= sc*128, and the one triangular 128x128 block is zeroed
    post-exp with a multiplicative bf16 mask (== -inf pre-exp). The PV matmul
    is split so unmasked columns don't wait for the mask.
  - PSUM accumulation note: only the first matmul of a reused PSUM tile may
    carry start=True (the first_mm flag clears the whole bank's has_written
    bits); later sub-range matmuls use start=False and rely on per-element
    has_written to overwrite-vs-accumulate.
  - Softmax denominator: running bf16 adds of exp blocks on DVE, one
    partition_all_reduce (Pool) + reciprocal + scale per (head, tb).
  - Startup: x^T tiles stream on the SP DMA queue while w streams in 2-chunk
    pieces on the ACT DMA queue just ahead of the first k-loop; mask and wo
    load later, off the startup critical path. Output staged to bf16 SBUF
    (DVE) and DMA'd per 512-column group; host sums partials in f32.
"""

import math

import numpy as np

import concourse.bass as bass
import concourse.bacc as bacc
import concourse.mybir as mybir
from concourse import bass_isa
from concourse.bass_utils import run_bass_kernel_spmd
from concourse.masks import make_identity
from concourse.tile import TileContext

F32 = mybir.dt.float32
BF16 = mybir.dt.bfloat16

D_MODEL = 2048
T = 2048
B = 2
N_HEADS = 16
N_KV = 4
HEAD_DIM = 128
GH = N_HEADS // N_KV  # 4 q heads per core
HALF = HEAD_DIM // 2
KD = D_MODEL // 128   # 16 contraction chunks
TC = T // 128         # 16 t-chunks of 128
TB = T // 512         # 4 t-blocks of 512


def build_nc(debug=False) -> bass.Bass:
    nc = bacc.Bacc("TRN2", target_bir_lowering=False)
    nc._marks = []

    def mark(label):
        nc._marks.append((label, int(nc.next_id())))

    # DRAM parameters (host supplies pre-tiled bf16 layouts; see kernel()).
    xt = nc.declare_dram_parameter("xt", [TC, 128, KD, 128], BF16, isOutput=False)
    w = nc.declare_dram_parameter("w", [128, KD, GH * 128 + 256], BF16, isOutput=False)
    wo = nc.declare_dram_parameter("wo", [128, GH, D_MODEL], BF16, isOutput=False)
    cs = nc.declare_dram_parameter("cs", [128, TC, 128], BF16, isOutput=False)
    gm = nc.declare_dram_parameter("gm", [128, 1024], BF16, isOutput=False)
    out = nc.declare_dram_parameter("out", [T, D_MODEL], BF16, isOutput=True)

    with TileContext(nc) as tc:
        with (
            tc.tile_pool(name="persist", bufs=1) as persist,
            tc.tile_pool(name="xtp", bufs=4) as xtp,
            tc.tile_pool(name="ropedst", bufs=3) as ropedst,
            tc.tile_pool(name="ropetmp", bufs=3) as ropetmp,
            tc.tile_pool(name="ptp", bufs=7) as ptpool,
            tc.tile_pool(name="laccp", bufs=2) as laccp,
            tc.tile_pool(name="lrepp", bufs=2) as lrepp,
            tc.tile_pool(name="otp", bufs=10) as otpool,
            tc.tile_pool(name="stagep", bufs=6) as stagep,
            tc.tile_pool(name="pqo", bufs=2, space="PSUM") as pqop,
            tc.tile_pool(name="pst", bufs=4, space="PSUM") as pstp,
            tc.tile_pool(name="pot", bufs=2, space="PSUM") as potp,
        ):
            # ---- resident tensors -------------------------------------
            W = persist.tile([128, KD, 768], BF16)
            WO = persist.tile([128, GH, D_MODEL], BF16)
            CS = persist.tile([128, TC, 128], BF16)
            GB = persist.tile([128, 1024], BF16)
            ident = persist.tile([128, 128], BF16)
            QTB = [persist.tile([128, GH, 512], BF16, name=f"qtb{tb}")
                   for tb in range(TB)]
            KTs = [persist.tile([128, 128], BF16, name=f"kt{s}")
                   for s in range(TC)]
            Vs = [persist.tile([128, 128], BF16, name=f"v{s}")
                  for s in range(TC)]

            # xt(0)/xt(1) first so the first projection can start ~6us in;
            # W in halves so chunks 0-7 don't wait for the full 24KB/partition.
            xt_pre = {}

            def issue_xt(t):
                tile = xtp.tile([128, KD, 128], BF16, tag="xt", name=f"xt{t}")
                nc.sync.dma_start(out=tile,
                                  in_=xt[t].rearrange("p k c -> p (k c)"))
                xt_pre[t] = tile

            # xt stream on the SP queue; persistent weights on the ACT hwdge
            # queue in 2-chunk pieces that stream just ahead of the first
            # projection's k-loop.
            nc.scalar.dma_start(out=W[:, 0:2, :],
                                in_=w[:, 0:2, :].rearrange("p k c -> p (k c)"))
            issue_xt(0)
            for kp in range(1, 8):
                nc.scalar.dma_start(
                    out=W[:, 2 * kp:2 * kp + 2, :],
                    in_=w[:, 2 * kp:2 * kp + 2, :].rearrange("p k c -> p (k c)"))
            issue_xt(1)
            nc.scalar.dma_start(out=CS, in_=cs.rearrange("p k c -> p (k c)"))
            issue_xt(2)
            issue_xt(3)
            # warm the PE HAM clock gate during the initial DMA wait: ~4us of
            # throwaway matmuls on a zeroed scratch tile so the first real
            # projection runs at 2.4GHz instead of 1.2GHz
            warm_src = persist.tile([128, 128], BF16, name="warm_src")
            nc.gpsimd.memset(warm_src, 0.0)
            make_identity(nc, ident)
            for wi in range(3):
                warm_ps = pstp.tile([128, 512], F32, tag="st",
                                    name=f"warm{wi}")
                for wj in range(10):
                    nc.tensor.matmul(warm_ps[:, 0:128], warm_src, warm_src,
                                     start=True, stop=True)

            def late_dmas(tb):
                # GB first needed by attn(0) (~45us), WO by out0 (~60us);
                # keep them off the DMA device during the startup crunch
                if tb == 1:
                    nc.scalar.dma_start(out=GB, in_=gm[:, :])
                elif tb == 2:
                    nc.scalar.dma_start(out=WO,
                                        in_=wo.rearrange("p h c -> p (h c)"))

            # transposes are deferred one chunk so PE never waits on rope
            pending = []

            def flush_pending():
                while pending:
                    t, dst = pending.pop(0)
                    mark(f"tp{t}")
                    tb, i = t // 4, t % 4
                    tp = pstp.tile([128, 640], BF16, tag="st",
                                   name=f"tp{t}")
                    for j in range(5):
                        nc.tensor.transpose(
                            tp[:, j * 128:(j + 1) * 128],
                            dst[:, j * 128:(j + 1) * 128], ident)
                    # one strided copy scatters the 4 roped-q transposes into
                    # QTB[tb][:, h, i*128:(i+1)*128] for all h at once
                    qdst = QTB[tb][:, :, i * 128:(i + 1) * 128]
                    qsrc = tp[:, 0:512].rearrange("p (h c) -> p h c", c=128)
                    nc.scalar.copy(qdst, qsrc)
                    nc.scalar.copy(KTs[t], tp[:, 512:640])

            def proj_iter(t, pq=None, pkv=None, xt_t=None):
                mark(f"proj{t}")
                if xt_t is None:
                    if t in xt_pre:
                        xt_t = xt_pre.pop(t)
                    else:
                        xt_t = xtp.tile([128, KD, 128], BF16, tag="xt",
                                        name=f"xt{t}")
                        nc.sync.dma_start(out=xt_t,
                                          in_=xt[t].rearrange("p k c -> p (k c)"))
                if pkv is None:
                    pkv = pstp.tile([128, 256], F32, tag="st",
                                    name=f"pkv{t}")
                    if pq is None:
                        pq = pqop.tile([128, 512], F32, tag="pq",
                                       name=f"pq{t}")
                        for k in range(KD):
                            lhs = xt_t[:, k, :]
                            nc.tensor.matmul(pq, lhs, W[:, k, 0:512],
                                             start=(k == 0),
                                             stop=(k == KD - 1))
                            nc.tensor.matmul(pkv, lhs, W[:, k, 512:768],
                                             start=(k == 0),
                                             stop=(k == KD - 1))
                    else:
                        # fused-startup path: q matmuls already issued
                        for k in range(KD):
                            nc.tensor.matmul(pkv, xt_t[:, k, :],
                                             W[:, k, 512:768],
                                             start=(k == 0),
                                             stop=(k == KD - 1))
                # stage pq/pkv to SBUF bf16 on ACT: frees the PSUM banks
                # early and lets rope run all-bf16 on DVE at 2x rate
                qc = ropetmp.tile([128, 512], BF16, tag="qc", name=f"qc{t}")
                kc = ropetmp.tile([128, 128], BF16, tag="kc", name=f"kc{t}")
                nc.scalar.copy(qc, pq)
                nc.scalar.copy(kc, pkv[:, 0:128])
                nc.scalar.copy(Vs[t], pkv[:, 128:256])
                # rope (q: 4 heads batched as 3D; k: single head) on DVE
                dst = ropedst.tile([128, 640], BF16, tag="rd", name=f"rd{t}")
                dst3 = dst.rearrange("p (h c) -> p h c", c=128)
                qc3 = qc.rearrange("p (h c) -> p h c", c=128)
                cosb = CS[:, t, None, 0:HALF].to_broadcast((128, GH, HALF))
                sinb = CS[:, t, None, HALF:128].to_broadcast((128, GH, HALF))
                q1, q2 = qc3[:, :, 0:HALF], qc3[:, :, HALF:128]
                t1 = ropetmp.tile([128, GH, HALF], BF16, tag="rt")
                t2 = ropetmp.tile([128, GH, HALF], BF16, tag="rt")
                nc.vector.tensor_mul(t1, q1, cosb)
                nc.vector.tensor_mul(t2, q2, sinb)
                nc.vector.tensor_sub(dst3[:, 0:GH, 0:HALF], t1, t2)
                t3 = ropetmp.tile([128, GH, HALF], BF16, tag="rt")
                t4 = ropetmp.tile([128, GH, HALF], BF16, tag="rt")
                nc.vector.tensor_mul(t3, q2, cosb)
                nc.vector.tensor_mul(t4, q1, sinb)
                nc.vector.tensor_add(dst3[:, 0:GH, HALF:128], t3, t4)
                cos2, sin2 = CS[:, t, 0:HALF], CS[:, t, HALF:128]
                k1, k2 = kc[:, 0:HALF], kc[:, HALF:128]
                t5 = ropetmp.tile([128, HALF], BF16, tag="rk")
                t6 = ropetmp.tile([128, HALF], BF16, tag="rk")
                nc.vector.tensor_mul(t5, k1, cos2)
                nc.vector.tensor_mul(t6, k2, sin2)
                nc.vector.tensor_sub(dst[:, 512:576], t5, t6)
                t7 = ropetmp.tile([128, HALF], BF16, tag="rk")
                t8 = ropetmp.tile([128, HALF], BF16, tag="rk")
                nc.vector.tensor_mul(t7, k2, cos2)
                nc.vector.tensor_mul(t8, k1, sin2)
                nc.vector.tensor_add(dst[:, 576:640], t7, t8)
                flush_pending()
                pending.append((t, dst))

            # outproj chunks from block tb-1 are interleaved between the
            # attention heads of block tb: they have no dependency on the
            # exp/softmax chain, so they keep PE busy while ACT catches up.
            pending_out = []

            def outproj_ngroup(tb, i, n, ots):
                t = 4 * tb + i
                mark(f"out{tb}_{i}")
                po = pqop.tile([128, 512], F32, tag="pq",
                               name=f"po{t}_{n}")
                for h in range(GH):
                    nc.tensor.matmul(po, ots[h][:, i * 128:(i + 1) * 128],
                                     WO[:, h, n * 512:(n + 1) * 512],
                                     start=(h == 0), stop=(h == GH - 1))
                stage = stagep.tile([128, 512], BF16, tag="sg",
                                    name=f"sg{t}_{n}")
                # gpsimd can't read PSUM; DVE has more slack than ACT
                nc.vector.tensor_copy(stage, po)
                nc.sync.dma_start(
                    out=out[t * 128:(t + 1) * 128, n * 512:(n + 1) * 512],
                    in_=stage)

            def outproj_chunk(tb, i, ots):
                for n in range(4):
                    outproj_ngroup(tb, i, n, ots)

            def attn_block(tb):
                nsc = 4 * (tb + 1)
                ots = []
                for h in range(GH):
                    mark(f"attn{tb}h{h}")
                    ot_ps = potp.tile([128, 512], F32, tag="ot",
                                      name=f"otp{h}_{tb}")
                    lacc = laccp.tile([128, 512], BF16, tag="lacc",
                                      name=f"la{h}_{tb}")
                    for sc in range(nsc):
                        if sc and sc % 4 == 0 and pending_out:
                            outproj_ngroup(*pending_out.pop(0))
                        # diagonal blocks: only compute t >= sc*128 (causal);
                        # strip j covers t in [j*128, 512)
                        j = sc - 4 * tb
                        off = j * 128 if j > 0 else 0
                        wd = 512 - off
                        st = pstp.tile([128, 512], F32, tag="st",
                                       name=f"st{h}_{tb}_{sc}")
                        nc.tensor.matmul(st[:, 0:wd], KTs[sc],
                                         QTB[tb][:, h, off:512],
                                         start=True, stop=True)
                        pt = ptpool.tile([128, 512], BF16, tag="pt",
                                         name=f"pt{h}_{tb}_{sc}")
                        nc.scalar.activation(pt[:, 0:wd], st[:, 0:wd],
                                             mybir.ActivationFunctionType.Exp)
                        if j >= 0:
                            # true diagonal 128-block: lower-triangular zeroing
                            nc.vector.tensor_mul(pt[:, 0:128], pt[:, 0:128],
                                                 GB[:, 384:512])
                        if j >= 0 and wd > 128:
                            # split PV: the unmasked columns don't wait for
                            # the diagonal mask mul. Only the FIRST matmul of
                            # a reused PSUM tile may carry start=True: the
                            # first_mm flag clears the whole bank's
                            # has_written bits, so a second start=True would
                            # wipe the first piece. Unwritten elements are
                            # overwritten (not accumulated) via per-element
                            # has_written, so start=False is correct for the
                            # second piece even at sc==0.
                            nc.tensor.matmul(ot_ps[:, off + 128:512], Vs[sc],
                                             pt[:, 128:wd],
                                             start=(sc == 0),
                                             stop=(sc == nsc - 1),
                                             skip_group_check=True)
                            nc.tensor.matmul(ot_ps[:, off:off + 128], Vs[sc],
                                             pt[:, 0:128],
                                             start=False,
                                             stop=(sc == nsc - 1),
                                             skip_group_check=True)
                        else:
                            nc.tensor.matmul(ot_ps[:, off:512], Vs[sc],
                                             pt[:, 0:wd],
                                             start=(sc == 0),
                                             stop=(sc == nsc - 1),
                                             skip_group_check=(j >= 0))
                        if sc == 0:
                            nc.vector.tensor_copy(lacc, pt)
                        else:
                            nc.vector.tensor_add(lacc[:, off:512],
                                                 lacc[:, off:512], pt[:, 0:wd])
                    lrep = lrepp.tile([128, 512], F32, tag="lr",
                                      name=f"lr{h}_{tb}")
                    nc.gpsimd.partition_all_reduce(
                        lrep, lacc, 128, bass_isa.ReduceOp.add)
                    nc.vector.reciprocal(lrep, lrep)
                    ot = otpool.tile([128, 512], BF16, tag="otb",
                                     name=f"ot{h}_{tb}")
                    nc.vector.tensor_mul(ot, ot_ps, lrep)
                    ots.append(ot)
                    if pending_out:
                        outproj_ngroup(*pending_out.pop(0))
                for i in range(4):
                    for n in range(4):
                        pending_out.append((tb, i, n, ots))

            # ---- software-pipelined main loop ------------------------
            for tb in range(TB):
                late_dmas(tb)
                if tb == 0:
                    # startup is W-DMA-paced: interleave the first two
                    # iterations' Q matmuls so each 2-chunk W piece is used
                    # twice as it arrives
                    xt_a, xt_b = xt_pre.pop(0), xt_pre.pop(1)
                    xt_c = xt_pre.pop(2)
                    pq_a = pqop.tile([128, 512], F32, tag="pq", name="pq0")
                    pq_b = pqop.tile([128, 512], F32, tag="pq", name="pq1")
                    pkv_a = pstp.tile([128, 256], F32, tag="st", name="pkv0")
                    # iter1's pkv and iter2's pq borrow idle pst/pot banks
                    pkv_b = pstp.tile([128, 256], F32, tag="st", name="pkv1")
                    pq_c = potp.tile([128, 512], F32, tag="ot", name="pq2")
                    for k in range(KD):
                        nc.tensor.matmul(pq_a, xt_a[:, k, :], W[:, k, 0:512],
                                         start=(k == 0), stop=(k == KD - 1))
                        nc.tensor.matmul(pkv_a, xt_a[:, k, :],
                                         W[:, k, 512:768],
                                         start=(k == 0), stop=(k == KD - 1))
                        nc.tensor.matmul(pq_b, xt_b[:, k, :], W[:, k, 0:512],
                                         start=(k == 0), stop=(k == KD - 1))
                        nc.tensor.matmul(pkv_b, xt_b[:, k, :],
                                         W[:, k, 512:768],
                                         start=(k == 0), stop=(k == KD - 1))
                        if k >= 6:
                            # iter2's Q joins once xt2 has landed (~6.6us);
                            # its weight chunk k-6 arrived long ago
                            nc.tensor.matmul(pq_c, xt_c[:, k - 6, :],
                                             W[:, k - 6, 0:512],
                                             start=(k == 6), stop=False)
                    for kc in range(10, KD):
                        nc.tensor.matmul(pq_c, xt_c[:, kc, :],
                                         W[:, kc, 0:512],
                                         start=False, stop=(kc == KD - 1))
                    proj_iter(0, pq=pq_a, pkv=pkv_a, xt_t=xt_a)
                    proj_iter(1, pq=pq_b, pkv=pkv_b, xt_t=xt_b)
                    proj_iter(2, pq=pq_c, xt_t=xt_c)
                    proj_iter(3)
                else:
                    for i in range(4):
                        proj_iter(4 * tb + i)
                if tb > 0:
                    attn_block(tb - 1)
            flush_pending()
            attn_block(TB - 1)
            while pending_out:
                outproj_ngroup(*pending_out.pop(0))

    nc.compile()
    return nc


def _np_bf16():
    import ml_dtypes
    return np.dtype(ml_dtypes.bfloat16)


def _prep_core_inputs(x_b, wq, wk, wv, wo, cs_cat, gmask, g):
    bf16 = _np_bf16()
    scale = 1.0 / math.sqrt(HEAD_DIM)
    wq_g = wq[:, g * 512:(g + 1) * 512] * scale
    wk_g = wk[:, g * 128:(g + 1) * 128]
    wv_g = wv[:, g * 128:(g + 1) * 128]
    wqkv = np.concatenate([wq_g, wk_g, wv_g], axis=1)          # [D, 768]
    w_t = np.ascontiguousarray(wqkv.reshape(KD, 128, 768).transpose(1, 0, 2))
    wo_g = wo[g * 512:(g + 1) * 512, :]                         # [512, D]
    wo_t = np.ascontiguousarray(wo_g.reshape(GH, 128, D_MODEL).transpose(1, 0, 2))
    xt = np.ascontiguousarray(
        x_b.reshape(TC, 128, KD, 128).transpose(0, 3, 2, 1))    # [tc,ki,ko,j]
    return {
        "xt": xt.astype(bf16),
        "w": w_t.astype(bf16),
        "wo": wo_t.astype(bf16),
        "cs": cs_cat.astype(bf16),
        "gm": gmask.astype(bf16),
    }


def _host_prep(x, wq, wk, wv, wo, cos, sin):
    cs = np.concatenate([cos, sin], axis=1)                     # [T, 128]
    cs_t = np.ascontiguousarray(
        cs.reshape(TC, 128, 128).transpose(1, 0, 2)).astype(np.float32)
    gmask = np.where(
        np.arange(1024)[None, :] >= np.arange(128)[:, None] + 384,
        np.float32(1.0), np.float32(0.0)).astype(np.float32)
    return cs_t, gmask


def kernel(x, wq, wk, wv, wo, cos, sin):
    x = np.asarray(x, np.float32)
    wq = np.asarray(wq, np.float32)
    wk = np.asarray(wk, np.float32)
    wv = np.asarray(wv, np.float32)
    wo = np.asarray(wo, np.float32)
    cos = np.asarray(cos, np.float32)
    sin = np.asarray(sin, np.float32)

    cs_t, gmask = _host_prep(x, wq, wk, wv, wo, cos, sin)

    nc = build_nc()
    in_maps = []
    for i in range(8):
        b, g = i // 4, i % 4
        in_maps.append(_prep_core_inputs(x[b], wq, wk, wv, wo, cs_t, gmask, g))

    try:
        res = run_bass_kernel_spmd(nc, in_maps, list(range(8)))
    except Exception:
        # a wedged NeuronCore can fail one execution with
        # NRT_EXEC_UNIT_UNRECOVERABLE and succeed on retry
        res = run_bass_kernel_spmd(nc, in_maps, list(range(8)))
    outs = [np.asarray(res.results[i]["out"], np.float32) for i in range(8)]
    full = np.empty((B, T, D_MODEL), np.float32)
    for b in range(B):
        full[b] = outs[4 * b] + outs[4 * b + 1] + outs[4 * b + 2] + outs[4 * b + 3]
    return full
